# revision 6
# baseline (speedup 1.0000x reference)
"""Trainium2 Bass kernel for a causal MHA block with RoPE (nn_CustomMHA).

Full-input contract: kernel(**inputs) takes the complete x/qkv/wo arrays,
shards them across 8 NeuronCores internally (head-group x batch), runs one
SPMD Bass program, and reassembles the full output on the host.

Sharding: core c => head group g = c // 4 (8 of 16 heads), batch b = c % 4.
Each core computes QKV projection + RoPE + causal attention + the wo
projection restricted to its head group's columns; the host sums the two
head-group partial outputs per batch (the "all-reduce" of the tensor-parallel
split) while unsharding.

Layout notes (everything feature-major / transposed so the PE contracts on
partitions):
  xT   [D, S]   = x[b].T          (fp32r)
  Q^T/K^T [512, S] per group      (bf16, RoPE applied)
  V65  [S/128, 128, 8*65]         (fp32r; per-head 65-wide slot, col 64 = 1.0
                                   so attn@V also accumulates the row sums)
  scores^T psum [128 keys, 512 queries] -> exp on ScalarE -> p (fp32r)
  attnout^T [512, S] fp32r -> woT matmuls -> out^T [D, S] partial
"""

import math

import numpy as np

D_MODEL = 1024
N_HEADS = 16
DH = 64
THETA = 10000.0
B_GLOB = 4
S_GLOB = 2048
N_CORES = 8
HG = 8            # heads per core (group)
FG = HG * DH      # feature columns per group = 512
QB = 512          # query block (max fp32 PSUM bank width)
KT = 128          # key tile (psum partitions)


def build_nc(S=S_GLOB, num_devices=N_CORES):
    """Build the per-core SPMD Bass program (same program on every core)."""
    import concourse.bacc as bacc
    import concourse.mybir as mybir
    import concourse.tile as tile

    F32 = mybir.dt.float32
    F32R = mybir.dt.float32r
    BF16 = mybir.dt.bfloat16
    Exp = mybir.ActivationFunctionType.Exp

    n_qb = S // QB          # query blocks
    n_st = S // KT          # seq tiles of 128
    n_j = D_MODEL // 128    # contraction tiles over D

    nc = bacc.Bacc("TRN2", target_bir_lowering=False, debug=False,
                   num_devices=num_devices)

    xT = nc.dram_tensor("xT", [D_MODEL, S], F32R, kind="ExternalInput")
    wqk = nc.dram_tensor("wqk", [D_MODEL, 2 * FG], F32R, kind="ExternalInput")
    wv = nc.dram_tensor("wv", [D_MODEL, FG], F32R, kind="ExternalInput")
    woT = nc.dram_tensor("woT", [FG, D_MODEL], F32R, kind="ExternalInput")
    cosT = nc.dram_tensor("cosT", [128, S], F32, kind="ExternalInput")
    sinTs = nc.dram_tensor("sinTs", [128, S], F32, kind="ExternalInput")
    mwide = nc.dram_tensor("mwide", [128, QB + 384], F32, kind="ExternalInput")
    outT = nc.dram_tensor("outT", [D_MODEL, S], F32, kind="ExternalOutput")

    with tile.TileContext(nc) as tc:
        from contextlib import ExitStack
        with ExitStack() as ctx:
            persist = ctx.enter_context(tc.tile_pool(name="persist", bufs=1))
            tmp_p = ctx.enter_context(tc.tile_pool(name="tmp_p", bufs=3))

            QT_sb = persist.tile([128, FG // 128, S], BF16, tag="QT")
            KT_sb = persist.tile([128, FG // 128, S], BF16, tag="KT")
            V65_sb = persist.tile([128, n_st, HG * 65], F32R, tag="V65")

            # ---------------- phase 1: projections (xT resident) -----------
            ctx1 = ExitStack()
            pxT = ctx1.enter_context(tc.tile_pool(name="pxT", bufs=1))
            xT_sb = pxT.tile([128, n_j, S], F32R, tag="xT")

            # ---------------- phase 1a: Q^T, K^T projection + RoPE ---------
            with tc.tile_pool(name="p1a", bufs=1) as p1a, \
                 tc.tile_pool(name="ps1", bufs=4, space="PSUM") as psum_p:
                for j in range(n_j):
                    nc.sync.dma_start(xT_sb[:, j, :], xT[j * 128:(j + 1) * 128, :])
                wqk_sb = p1a.tile([128, n_j, 2 * FG], F32R, tag="wqk")
                for j in range(n_j):
                    nc.sync.dma_start(wqk_sb[:, j, :], wqk[j * 128:(j + 1) * 128, :])
                cos_sb = p1a.tile([128, S], F32, tag="cos")
                nc.sync.dma_start(cos_sb[:], cosT[:])
                sin_sb = p1a.tile([128, S], F32, tag="sin")
                nc.sync.dma_start(sin_sb[:], sinTs[:])

                for m in range(2 * FG // 128):      # 8 feature tiles (4 Q, 4 K)
                    for s in range(n_qb):
                        ps = psum_p.tile([128, QB], F32, tag="pp")
                        for j in range(n_j):
                            nc.tensor.matmul(
                                ps[:],
                                wqk_sb[:, j, m * 128:(m + 1) * 128],
                                xT_sb[:, j, s * QB:(s + 1) * QB],
                                start=(j == 0), stop=(j == n_j - 1))
                        cs = cos_sb[:, s * QB:(s + 1) * QB]
                        ss = sin_sb[:, s * QB:(s + 1) * QB]
                        t1 = tmp_p.tile([128, QB], F32, tag="t1")
                        nc.vector.tensor_mul(t1[:], ps[:], cs)
                        t2 = tmp_p.tile([128, QB], F32, tag="t2")
                        nc.vector.tensor_mul(t2[:], ps[:], ss)
                        t2w = tmp_p.tile([128, QB], F32, tag="t2w")
                        for o in (0, 32, 64, 96):
                            nc.sync.dma_start(t2w[o:o + 32, :],
                                              t2[o ^ 32:(o ^ 32) + 32, :])
                        dst = QT_sb if m < 4 else KT_sb
                        nc.vector.tensor_add(
                            dst[:, m % 4, s * QB:(s + 1) * QB], t1[:], t2w[:])

            # ---------------- phase 1b: V projection into 65-slot layout ---
            with tc.tile_pool(name="p1b", bufs=1) as p1b, \
                 tc.tile_pool(name="ps1b", bufs=4, space="PSUM") as psum_p:
                wv_sb = p1b.tile([128, n_j, FG], F32R, tag="wv")
                for j in range(n_j):
                    nc.sync.dma_start(wv_sb[:, j, :], wv[j * 128:(j + 1) * 128, :])
                for st in range(n_st):
                    nc.vector.memset(V65_sb[:, st, :].bitcast(F32), 1.0)
                    ps = psum_p.tile([128, FG], F32, tag="pp")
                    for j in range(n_j):
                        nc.tensor.matmul(
                            ps[:],
                            xT_sb[:, j, st * 128:(st + 1) * 128],
                            wv_sb[:, j, :],
                            start=(j == 0), stop=(j == n_j - 1))
                    nc.vector.tensor_copy(
                        V65_sb[:, st, :].rearrange("p (h x) -> p h x", x=65)[:, :, 0:64],
                        ps[:].rearrange("p (h x) -> p h x", x=64))
            ctx1.close()   # free xT

            # ---------------- phase 2: causal attention --------------------
            pattn = ctx.enter_context(tc.tile_pool(name="pattn", bufs=1))
            attnT_sb = pattn.tile([128, FG // 128, S], F32R, tag="attnT")
            with tc.tile_pool(name="p2", bufs=1) as p2, \
                 tc.tile_pool(name="p2p", bufs=4) as p2p, \
                 tc.tile_pool(name="ps2", bufs=4, space="PSUM") as psum_p, \
                 tc.tile_pool(name="p2av", bufs=3, space="PSUM") as p2av, \
                 tc.tile_pool(name="p2dr", bufs=4, space="DRAM") as p2dr:
                mw_sb = p2.tile([128, QB + 384], F32, tag="mw")
                nc.sync.dma_start(mw_sb[:], mwide[:])

                for h in range(HG):
                    p0 = (h % 2) * 64
                    f = h // 2
                    for qb in range(n_qb):
                        n_kt = (qb + 1) * (QB // KT)
                        pav = p2av.tile([65, QB], F32, tag="pav")
                        for kt in range(n_kt):
                            pscr = psum_p.tile([128, QB], F32, tag="ps")
                            nc.tensor.matmul(
                                pscr[:],
                                KT_sb[p0:p0 + 64, f, kt * KT:(kt + 1) * KT],
                                QT_sb[p0:p0 + 64, f, qb * QB:(qb + 1) * QB],
                                start=True, stop=True)
                            dd = kt - (qb * (QB // KT))
                            pt = p2p.tile([128, QB], F32R, tag="pt")
                            if dd >= 0:  # diagonal tile: mask after exp
                                pe = p2p.tile([128, QB], F32, tag="pe")
                                nc.scalar.activation(pe[:], pscr[:], Exp)
                                nc.vector.tensor_mul(
                                    pt[:], pe[:],
                                    mw_sb[:, 384 - 128 * dd:384 - 128 * dd + QB])
                            else:
                                nc.scalar.activation(pt[:], pscr[:], Exp)
                            nc.tensor.matmul(
                                pav[:],
                                V65_sb[:, kt, h * 65:(h + 1) * 65],
                                pt[:],
                                start=(kt == 0), stop=(kt == n_kt - 1))
                        # normalize: out[d, q] = pav[d, q] / pav[64, q]
                        # (partition-aligned ops only; DMA moves partitions)
                        srow = p2p.tile([128, QB], F32, tag="srow")
                        nc.vector.tensor_copy(srow[64:65, :], pav[64:65, :])
                        drow = p2dr.tile([1, QB], F32, tag="drow")
                        nc.sync.dma_start(drow[:], srow[64:65, :])
                        rb = p2p.tile([64, QB], F32, tag="rb")
                        nc.sync.dma_start(rb[:], drow[0:1, :].to_broadcast((64, QB)))
                        rrb = p2p.tile([64, QB], F32, tag="rrb")
                        nc.vector.reciprocal(rrb[:], rb[:])
                        stg = p2p.tile([64, QB], F32R, tag="stg")
                        nc.vector.tensor_mul(stg[:], pav[0:64, :], rrb[:])
                        nc.sync.dma_start(
                            attnT_sb[p0:p0 + 64, f, qb * QB:(qb + 1) * QB], stg[:])

            # ---------------- phase 3: wo projection (partial out^T) -------
            with tc.tile_pool(name="p3", bufs=1) as p3, \
                 tc.tile_pool(name="ps3", bufs=4, space="PSUM") as psum_p, \
                 tc.tile_pool(name="p3o", bufs=4) as p3o:
                woT_sb = p3.tile([128, FG // 128, D_MODEL], F32R, tag="woT")
                for j in range(FG // 128):
                    nc.sync.dma_start(woT_sb[:, j, :], woT[j * 128:(j + 1) * 128, :])
                for m in range(n_j):
                    for s in range(n_qb):
                        ps = psum_p.tile([128, QB], F32, tag="po")
                        for j in range(FG // 128):
                            nc.tensor.matmul(
                                ps[:],
                                woT_sb[:, j, m * 128:(m + 1) * 128],
                                attnT_sb[:, j, s * QB:(s + 1) * QB],
                                start=(j == 0), stop=(j == FG // 128 - 1))
                        so = p3o.tile([128, QB], F32, tag="so")
                        nc.vector.tensor_copy(so[:], ps[:])
                        nc.sync.dma_start(
                            outT[m * 128:(m + 1) * 128, s * QB:(s + 1) * QB], so[:])

    nc.compile()
    return nc


def make_tables(S=S_GLOB):
    """Host-side RoPE tables + diagonal causal mask, in kernel layout."""
    inv_freq = 1.0 / (THETA ** (np.arange(0, DH, 2, dtype=np.float64) / DH))
    ang = np.arange(S, dtype=np.float64)[:, None] * inv_freq[None, :]  # [S, 32]
    cos64 = np.concatenate([np.cos(ang), np.cos(ang)], axis=1)  # [S, 64]
    sin32 = np.sin(ang)                                         # [S, 32]
    p = np.arange(128)
    d = p % 64
    cosT = cos64[:, d].T.astype(np.float32)                     # [128, S]
    sign = np.where(d < 32, -1.0, 1.0)
    sinT_signed = (sign[:, None] * sin32[:, p % 32].T).astype(np.float32)
    sinTs = sinT_signed[p ^ 32, :]                              # swap-folded
    # mwide[r, c] = 1 iff c >= r + 384   (diagonal-tile causal masks)
    r = np.arange(128)[:, None]
    c = np.arange(QB + 384)[None, :]
    mwide = (c >= r + 384).astype(np.float32)
    return np.ascontiguousarray(cosT), np.ascontiguousarray(sinTs), mwide


def make_in_maps(x, qkv, wo, S=S_GLOB):
    """Shard full inputs into one input map per core."""
    x = np.asarray(x, dtype=np.float32)
    qkv = np.asarray(qkv, dtype=np.float32)
    wo = np.asarray(wo, dtype=np.float32)
    cosT, sinTs, mwide = make_tables(S)
    scale = 1.0 / math.sqrt(float(DH))
    in_maps = []
    for c in range(N_CORES):
        g, b = c // 4, c % 4
        rows = slice(g * FG, (g + 1) * FG)
        wq = qkv[0 * D_MODEL:1 * D_MODEL][rows] * scale   # [512, 1024]
        wk = qkv[1 * D_MODEL:2 * D_MODEL][rows]
        wv_ = qkv[2 * D_MODEL:3 * D_MODEL][rows]
        in_maps.append({
            "xT": np.ascontiguousarray(x[b].T),
            "wqk": np.ascontiguousarray(np.concatenate([wq, wk], axis=0).T),
            "wv": np.ascontiguousarray(wv_.T),
            "woT": np.ascontiguousarray(wo[:, rows].T),
            "cosT": cosT,
            "sinTs": sinTs,
            "mwide": mwide,
        })
    return in_maps


def assemble_output(results, S=S_GLOB):
    """Sum head-group partials per batch and transpose back to [B, S, D]."""
    out = np.empty((B_GLOB, S, D_MODEL), dtype=np.float32)
    for b in range(B_GLOB):
        acc = results[b]["outT"] + results[4 + b]["outT"]
        out[b] = acc.T
    return out


_NC_CACHE = {}


def kernel(x, qkv, wo):
    from concourse.bass_utils import run_bass_kernel_spmd
    if "nc" not in _NC_CACHE:
        _NC_CACHE["nc"] = build_nc()
    nc = _NC_CACHE["nc"]
    in_maps = make_in_maps(x, qkv, wo)
    res = run_bass_kernel_spmd(nc, in_maps, list(range(N_CORES)))
    return assemble_output(res.results)
